# revision 1
# baseline (speedup 1.0000x reference)
"""CCX loss kernel for Trainium2 (8 NeuronCores, data-parallel over batch).

Math (per batch element n, with C=256 channels, HW=64*64=4096 pixels):
  y_mu[c]   = mean over (n, h, w) of y            (host, tiny)
  x_c = x - y_mu ; y_c = y - y_mu                 (device)
  x_n = x_c/||x_c||_C ; y_n = y_c/||y_c||_C       (device)
  s[i,j]    = sum_c x_n[c,i] y_n[c,j]             (device matmul, f32r)
  d = 1-s ; dt = d/(dmin_i+eps) ; w = exp((1-dt)/0.5)
  ccx_ij = w/sum_j w ; ccx_n = mean_j max_i ccx_ij
  loss = mean_n -log(ccx_n + eps)                 (host, 8 scalars)

Key identities used on device:
  w_ij = exp(s*a_i + b_i),  a_i = 2/(dmin_i+eps), b_i = 2-a_i
  s*a_i = G*alpha_i with G = x_c^T y_n (unnormalized-x matmul),
          alpha_i = a_i/||x_c[:,i]||
  max_i ccx_ij = exp(max_i (G^T[j,i]*alpha_i + (b_i - lnZ_i)))
  -> pass 2 computes A = (x_c*alpha)^T-style matmul + K=1 bias-row matmul,
     then a plain free-dim max per j-block.
"""

import os
import sys

import numpy as np

sys.path.insert(0, "/opt/trn_rl_repo")
os.environ.setdefault("JAX_PLATFORMS", "axon")

import concourse.mybir as mybir
import concourse.tile as tile
from concourse import bacc, bass_isa
from concourse.bass_utils import run_bass_kernel_spmd
from concourse.masks import make_identity

N, C, H, W = 8, 256, 64, 64
HW = H * W          # 4096
NB = HW // 128      # 32 blocks of 128 rows/cols
NQ = 4              # psum quarters per block
QW = HW // NB // 1  # 128
QUARTER = 1024      # quarter width (2 psum banks)
EPS = 1e-6
F32 = mybir.dt.float32
F32R = mybir.dt.float32r
ALU = mybir.AluOpType
ACTF = mybir.ActivationFunctionType

_cached = {}


def _build():
    nc = bacc.Bacc(None, target_bir_lowering=False, debug=True)
    xs = nc.dram_tensor("xs", [C, HW], F32, kind="ExternalInput")
    ys = nc.dram_tensor("ys", [C, HW], F32, kind="ExternalInput")
    ymu = nc.dram_tensor("ymu", [128, 2], F32, kind="ExternalInput")
    out = nc.dram_tensor("out", [1, 1], F32, kind="ExternalOutput")
    scr_y = nc.dram_tensor("scr_y", [NB, 128], F32)
    scr_a = nc.dram_tensor("scr_a", [NB, 128], F32)
    scr_b = nc.dram_tensor("scr_b", [NB, 128], F32)

    repeat = int(os.environ.get("BASS_REPEAT", "1"))
    phase = int(os.environ.get("BASS_PHASE", "3"))
    with tile.TileContext(nc) as tc:
        import contextlib
        rep_ctx = tc.For_i(0, repeat, 1) if repeat > 1 else contextlib.nullcontext()
        with rep_ctx:
         with (
             tc.tile_pool(name="big", bufs=1) as big,
             tc.tile_pool(name="bc3", bufs=3) as bc3,
             tc.tile_pool(name="sq", bufs=2) as sqp,
             tc.tile_pool(name="small", bufs=1) as sm,
             tc.tile_pool(name="mmq", bufs=8, space="PSUM") as mmq,
         ):
             # ---------------- load ----------------
             x = big.tile([128, 2, HW], F32, tag="x")
             xc = big.tile([128, 2, HW], F32R, tag="xc")
             y = big.tile([128, 2, HW], F32, tag="y")
             yn = big.tile([128, 2, HW], F32R, tag="yn")
             ymu_sb = sm.tile([128, 2], F32, tag="ymu")
             nc.sync.dma_start(out=x[:, :, :], in_=xs.rearrange("(g p) j -> p g j", p=128))
             nc.sync.dma_start(out=y[:, :, :], in_=ys.rearrange("(g p) j -> p g j", p=128))
             nc.sync.dma_start(out=ymu_sb[:, :], in_=ymu[:, :])

             ones_col = sm.tile([128, 1], F32, tag="ones_col")
             nc.vector.memset(ones_col[:, :], 1.0)
             ones_row_f = sm.tile([1, 128], F32, tag="ones_row_f")
             nc.vector.memset(ones_row_f[:, :], 1.0)
             ones_row_r = sm.tile([1, 128], F32R, tag="ones_row_r")
             nc.vector.tensor_scalar(
                 out=ones_row_r[:, :], in0=ones_row_f[:, :], scalar1=1.0,
                 scalar2=None, op0=ALU.mult)

             # ---------------- center (in place; x rounds to f32r) --------
             for g in range(2):
                 nc.vector.tensor_scalar(
                     out=xc[:, g, :], in0=x[:, g, :],
                     scalar1=ymu_sb[:, g : g + 1], scalar2=None, op0=ALU.subtract)
                 nc.vector.tensor_scalar(
                     out=y[:, g, :], in0=y[:, g, :],
                     scalar1=ymu_sb[:, g : g + 1], scalar2=None, op0=ALU.subtract)

             # ---------------- channel sumsq -> 1/norm (col layout) -------
             # pscol[:, r] (x) / pscol[:, 32+r] (y): per-pixel sum over C of sq
             pscol = mmq.tile([128, 64], F32, tag="pq")
             for ti, src in ((0, xc), (1, y)):
                 for ch in range(4):
                     sqs = []
                     for g in range(2):
                         sq = sqp.tile([128, 1024], F32, tag="sqt")
                         nc.scalar.activation(
                             out=sq[:, :],
                             in_=src[:, g, 1024 * ch : 1024 * (ch + 1)].bitcast(F32),
                             func=ACTF.Square)
                         sqs.append(sq)
                     for k in range(8):
                         r = 8 * ch + k
                         for g in range(2):
                             nc.tensor.matmul(
                                 pscol[:, 32 * ti + r : 32 * ti + r + 1],
                                 sqs[g][:, 128 * k : 128 * (k + 1)],
                                 ones_col[:, :],
                                 start=(g == 0), stop=(g == 1))
             norms = sm.tile([128, 64], F32, tag="norms")
             nc.scalar.activation(out=norms[:, :], in_=pscol[:, :], func=ACTF.Sqrt)
             invc = sm.tile([128, 64], F32, tag="invc")
             nc.vector.reciprocal(invc[:, :], norms[:, :])
             # invx = invc[:, 0:32], invy = invc[:, 32:64]

             # ---------------- broadcast invy along partitions -------------
             # col->DRAM (transposing dst AP), then DRAM->SBUF partition-bcast
             import concourse.bass as bass_mod
             nc.sync.dma_start(
                 out=scr_y[:, :].rearrange("r p -> p r"), in_=invc[:, 32:64])
             invybc = bc3.tile([128, HW], F32, tag="bcast")
             bcast_src_y = bass_mod.AP(
                 tensor=scr_y[:, :].tensor, offset=0, ap=[[0, 128], [1, HW]])
             nc.sync.dma_start(out=invybc[:, :], in_=bcast_src_y)

             # ---------------- y_n = y_c * invy (f32r) ---------------------
             for g in range(2):
                 nc.vector.tensor_tensor(
                     out=yn[:, g, :], in0=y[:, g, :], in1=invybc[:, :],
                     op=ALU.mult)

             # ---------------- PASS 1: row max + Z -------------------------
             gacc = sm.tile([128, 256], F32, tag="gacc")
             zacc = sm.tile([128, 256], F32, tag="zacc")
             gmaxc = sm.tile([128, 32], F32, tag="gmaxc")
             reccol = sm.tile([128, 32], F32, tag="reccol")
             ab64 = sm.tile([128, 64], F32, tag="ab64")  # alpha | b2
             bcol = sm.tile([128, 32], F32, tag="bcol")
             tmpc = sm.tile([128, 32], F32, tag="tmpc")

             for r in range(NB if phase >= 1 else 0):
                 pqs = [mmq.tile([128, 512], F32, tag="pq", name=f"pq_{_i}") for _i in range(8)]
                 for cp in range(4):
                     c0, c1 = 2 * cp, 2 * cp + 1
                     for g in range(2):
                         for c in (c0, c1):
                             nc.tensor.matmul(
                                 pqs[c][:, :],
                                 xc[:, g, 128 * r : 128 * (r + 1)],
                                 yn[:, g, 512 * c : 512 * (c + 1)],
                                 start=(g == 0), stop=(g == 1))
                     for c in (c0, c1):
                         nc.vector.reduce_max(
                             gacc[:, 8 * r + c : 8 * r + c + 1], pqs[c][:, :],
                             axis=mybir.AxisListType.X)
                 # alpha chain for this block
                 nc.vector.reduce_max(
                     gmaxc[:, r : r + 1],
                     gacc[:, 8 * r : 8 * r + 8], axis=mybir.AxisListType.X)
                 # smax = gmax * invx_r ; dminE = 1+eps - smax
                 nc.vector.tensor_scalar(
                     out=tmpc[:, r : r + 1], in0=gmaxc[:, r : r + 1],
                     scalar1=invc[:, r : r + 1], scalar2=None, op0=ALU.mult)
                 nc.vector.tensor_scalar(
                     out=tmpc[:, r : r + 1], in0=tmpc[:, r : r + 1],
                     scalar1=-1.0, scalar2=float(1.0 + EPS),
                     op0=ALU.mult, op1=ALU.add)
                 nc.vector.reciprocal(reccol[:, r : r + 1], tmpc[:, r : r + 1])
                 nc.vector.tensor_scalar(
                     out=ab64[:, r : r + 1], in0=reccol[:, r : r + 1],
                     scalar1=invc[:, r : r + 1], scalar2=2.0,
                     op0=ALU.mult, op1=ALU.mult)
                 nc.vector.tensor_scalar(
                     out=bcol[:, r : r + 1], in0=reccol[:, r : r + 1],
                     scalar1=-2.0, scalar2=2.0, op0=ALU.mult, op1=ALU.add)
                 for c in range(8):
                     nc.scalar.activation(
                         out=pqs[c][:, :], in_=pqs[c][:, :], func=ACTF.Exp,
                         bias=bcol[:, r : r + 1],
                         scale=ab64[:, r : r + 1],
                         accum_out=zacc[:, 8 * r + c : 8 * r + c + 1])

             # ---------------- interlude: b2 = b1 - lnZ; broadcasts --------
             if phase >= 2:
              zsum = sm.tile([128, 32], F32, tag="zsum")
              nc.vector.reduce_sum(
                  zsum[:, :], zacc[:, :].rearrange("p (r q) -> p r q", q=8),
                  axis=mybir.AxisListType.X)
              lnz = sm.tile([128, 32], F32, tag="lnz")
              nc.scalar.activation(out=lnz[:, :], in_=zsum[:, :], func=ACTF.Ln)
              nc.vector.tensor_tensor(
                  out=ab64[:, 32:64], in0=bcol[:, :], in1=lnz[:, :], op=ALU.subtract)

              # alpha/b2 cols -> DRAM rows (transposing dst), then bcast/row
              nc.sync.dma_start(
                  out=scr_a[:, :].rearrange("r p -> p r"), in_=ab64[:, 0:32])
              nc.sync.dma_start(
                  out=scr_b[:, :].rearrange("r p -> p r"), in_=ab64[:, 32:64])
              abc = bc3.tile([128, HW], F32, tag="bcast")
              bcast_src_a = bass_mod.AP(
                  tensor=scr_a[:, :].tensor, offset=0, ap=[[0, 128], [1, HW]])
              nc.sync.dma_start(out=abc[:, :], in_=bcast_src_a)
              b2row_f = bc3.tile([1, HW], F32, tag="bcast")
              nc.sync.dma_start(
                  out=b2row_f[0:1, :],
                  in_=scr_b[:, :].rearrange("r p -> (r p)"))
              b2row = bc3.tile([1, HW], F32R, tag="bcast")
              nc.vector.tensor_scalar(
                  out=b2row[:, :], in0=b2row_f[:, :], scalar1=1.0, scalar2=None,
                  op0=ALU.mult)

              # x2 = x_c * alpha  (in place, f32r)
              for g in range(2):
                  nc.vector.tensor_tensor(
                      out=xc[:, g, :], in0=xc[:, g, :].bitcast(F32),
                      in1=abc[:, :], op=ALU.mult)

             # ---------------- PASS 2: col max of A ------------------------
             macc = sm.tile([128, 256], F32, tag="macc")
             if phase < 3:
                 nc.vector.memset(macc[:, :], -1.0)
             for rb in range(NB if phase >= 3 else 0):
                 pqs = [mmq.tile([128, 512], F32, tag="pq", name=f"pq_{_i}") for _i in range(8)]
                 for c in range(8):
                     nc.tensor.matmul(
                         pqs[c][:, :], ones_row_r[:, :],
                         b2row[:, 512 * c : 512 * (c + 1)],
                         start=True, stop=False)
                 for c in range(8):
                     nc.tensor.matmul(
                         pqs[c][:, :],
                         yn[:, 0, 128 * rb : 128 * (rb + 1)],
                         xc[:, 0, 512 * c : 512 * (c + 1)],
                         start=False, stop=False)
                 for c in range(8):
                     nc.tensor.matmul(
                         pqs[c][:, :],
                         yn[:, 1, 128 * rb : 128 * (rb + 1)],
                         xc[:, 1, 512 * c : 512 * (c + 1)],
                         start=False, stop=True)
                     nc.vector.reduce_max(
                         macc[:, 8 * rb + c : 8 * rb + c + 1], pqs[c][:, :],
                         axis=mybir.AxisListType.X)

             # ---------------- final ---------------------------------------
             mcol = sm.tile([128, 32], F32, tag="mcol")
             nc.vector.reduce_max(
                 mcol[:, :], macc[:, :].rearrange("p (r q) -> p r q", q=8),
                 axis=mybir.AxisListType.X)
             expm = sm.tile([128, 32], F32, tag="expm")
             csum = sm.tile([128, 1], F32, tag="csum")
             nc.scalar.activation(
                 out=expm[:, :], in_=mcol[:, :], func=ACTF.Exp,
                 accum_out=csum[:, :])
             tot = sm.tile([128, 1], F32, tag="tot")
             nc.gpsimd.partition_all_reduce(
                 tot[:, :], csum[:, :], channels=128,
                 reduce_op=bass_isa.ReduceOp.add)
             res = sm.tile([1, 1], F32, tag="res")
             nc.vector.tensor_scalar(
                 out=res[:, :], in0=tot[0:1, :], scalar1=float(1.0 / HW),
                 scalar2=None, op0=ALU.mult)
             nc.sync.dma_start(out=out[:, :], in_=res[:, :])
    nc.compile()
    return nc


def _get_nc():
    if "nc" not in _cached:
        _cached["nc"] = _build()
    return _cached["nc"]


def run_device(x, y, trace=False):
    """x, y: (N, C, H, W) float32. Returns (ccx (N,), BassKernelResults)."""
    x = np.ascontiguousarray(np.asarray(x, dtype=np.float32))
    y = np.ascontiguousarray(np.asarray(y, dtype=np.float32))
    ymu = y.mean(axis=(0, 2, 3), dtype=np.float64).astype(np.float32)  # (C,)
    ymu_arr = np.ascontiguousarray(ymu.reshape(2, 128).T)  # (128, 2)
    in_maps = []
    for n in range(N):
        in_maps.append({
            "xs": np.ascontiguousarray(x[n].reshape(C, HW)),
            "ys": np.ascontiguousarray(y[n].reshape(C, HW)),
            "ymu": ymu_arr,
        })
    nc = _get_nc()
    res = run_bass_kernel_spmd(nc, in_maps, core_ids=list(range(N)), trace=trace)
    ccx = np.array([res.results[n]["out"][0, 0] for n in range(N)], dtype=np.float32)
    return ccx, res


def kernel(x, y):
    ccx, _ = run_device(x, y)
    loss = float(np.mean(-np.log(ccx.astype(np.float64) + EPS)))
    return np.float32(loss)


if __name__ == "__main__":
    rng = np.random.default_rng(0)
    x = rng.standard_normal((N, C, H, W), dtype=np.float32)
    y = rng.standard_normal((N, C, H, W), dtype=np.float32)
    print("loss:", kernel(x, y))



# revision 15
# speedup vs baseline: 1.0078x; 1.0078x over previous
"""CCX loss kernel for Trainium2 (8 NeuronCores, data-parallel over batch).

Math (per batch element n, with C=256 channels, HW=64*64=4096 pixels):
  y_mu[c]   = mean over (n, h, w) of y            (host, tiny)
  x_c = x - y_mu ; y_c = y - y_mu                 (device)
  x_n = x_c/||x_c||_C ; y_n = y_c/||y_c||_C       (device)
  s[i,j]    = sum_c x_n[c,i] y_n[c,j]             (device matmul, f32r)
  d = 1-s ; dt = d/(dmin_i+eps) ; w = exp((1-dt)/0.5)
  ccx_ij = w/sum_j w ; ccx_n = mean_j max_i ccx_ij
  loss = mean_n -log(ccx_n + eps)                 (host, 8 scalars)

Key identities used on device:
  w_ij = exp(s*a_i + b_i),  a_i = 2/(dmin_i+eps), b_i = 2-a_i
  s*a_i = G*alpha_i with G = x_c^T y_n (unnormalized-x matmul),
          alpha_i = a_i/||x_c[:,i]||
  max_i ccx_ij = exp(max_i (G^T[j,i]*alpha_i + (b_i - lnZ_i)))
  -> pass 2: matmul G2 = yn^T x2 (x2 = x_c*alpha), then a single DVE
     tensor_tensor_reduce per psum tile adds the broadcast bias row
     b2_i and max-reduces over the free dim (i) in one 1x pass.

Perf structure (TRN2):
  - psum tiles are TW-wide (default 2048 = 4 banks), ring of 16KB/TW
    generations; matmuls write 512-wide slices (bank granularity).
  - pass1 per block: 2K-group matmuls share LDWEIGHTS (g outer),
    reduce_max per tile -> row stats chain (split DVE/ACT) -> exp
    in-place with Z accumulation (ACT).
  - pass2 per block: matmuls + fused bias+max TTR (no K=1 bias matmul).
"""

import os
import sys

import numpy as np

sys.path.insert(0, "/opt/trn_rl_repo")
os.environ.setdefault("JAX_PLATFORMS", "axon")

import concourse.mybir as mybir
import concourse.tile as tile
from concourse import bacc, bass_isa
from concourse.bass_utils import run_bass_kernel_spmd

N, C, H, W = 8, 256, 64, 64
HW = H * W          # 4096
EPS = 1e-6
F32 = mybir.dt.float32
F32R = mybir.dt.float32r
BF16 = mybir.dt.bfloat16
ALU = mybir.AluOpType
ACTF = mybir.ActivationFunctionType

TW = int(os.environ.get("BASS_TW", "2048"))    # psum tile width
NO_TTR = os.environ.get("BASS_NO_TTR", "0") == "1"
OLD_SUMSQ = os.environ.get("BASS_OLD_SUMSQ", "0") == "1"
ACT_CENTER = os.environ.get("BASS_ACT_CENTER", "1") == "1"
NTB = HW // TW                                  # tiles per block row
NBUF = 16384 // (TW * 4)                        # psum ring generations
NB = HW // 128                                  # 32 row/col blocks
NEG_INF = float(os.environ.get("BASS_NEGINF", "-1e30"))

_cached = {}


def _build():
    nc = bacc.Bacc(None, target_bir_lowering=False, debug=True)
    xs = nc.dram_tensor("xs", [C, HW], F32, kind="ExternalInput")
    ys = nc.dram_tensor("ys", [C, HW], F32, kind="ExternalInput")
    ymu = nc.dram_tensor("ymu", [128, 2], F32, kind="ExternalInput")
    out = nc.dram_tensor("out", [1, 1], F32, kind="ExternalOutput")
    scr_n = nc.dram_tensor("scr_n", [2, HW], F32)   # normsq rows (x, y)
    scr_y = nc.dram_tensor("scr_y", [NB, 128], F32)
    scr_a = nc.dram_tensor("scr_a", [NB, 128], F32)
    scr_b = nc.dram_tensor("scr_b", [NB, 128], F32)

    import concourse.bass as bass_mod

    with tile.TileContext(nc) as tc:
        with (
            tc.tile_pool(name="big", bufs=1) as big,
            tc.tile_pool(name="bc3", bufs=1) as bc3,
            tc.tile_pool(name="sq", bufs=2) as sqp,
            tc.tile_pool(name="small", bufs=1) as sm,
            tc.tile_pool(name="mmq", bufs=NBUF, space="PSUM") as mmq,
        ):
            # ---------------- load ----------------
            x = big.tile([128, 2, HW], F32, tag="x")
            y = big.tile([128, 2, HW], F32, tag="y")
            xc = big.tile([128, 2, HW], F32R, tag="xc")
            yn = big.tile([128, 2, HW], F32R, tag="yn")
            ymu_sb = sm.tile([128, 2], F32, tag="ymu")
            nc.sync.dma_start(out=x[:, :, :],
                              in_=xs.rearrange("(g p) j -> p g j", p=128))
            nc.sync.dma_start(out=y[:, :, :],
                              in_=ys.rearrange("(g p) j -> p g j", p=128))
            nc.sync.dma_start(out=ymu_sb[:, :], in_=ymu[:, :])

            ones_col = sm.tile([128, 1], BF16, tag="ones_col")
            nc.vector.memset(ones_col[:, :], 1.0)
            negymu = sm.tile([128, 2], F32, tag="negymu")
            nc.vector.tensor_scalar(
                out=negymu[:, :], in0=ymu_sb[:, :], scalar1=-1.0,
                scalar2=None, op0=ALU.mult)

            # ---------------- center (in place; rounds to f32r) ----------
            for g in range(2):
                # x on DVE (2x fp32 sbuf), y on ACT (Identity + bias)
                nc.vector.tensor_scalar(
                    out=xc[:, g, :], in0=x[:, g, :],
                    scalar1=ymu_sb[:, g : g + 1], scalar2=None,
                    op0=ALU.subtract)
                if ACT_CENTER:
                    nc.scalar.activation(
                        out=yn[:, g, :], in_=y[:, g, :],
                        func=ACTF.Identity, bias=negymu[:, g : g + 1],
                        scale=1.0)
                else:
                    nc.vector.tensor_scalar(
                        out=yn[:, g, :], in0=y[:, g, :],
                        scalar1=ymu_sb[:, g : g + 1], scalar2=None,
                        op0=ALU.subtract)

            # ---------------- channel sumsq -> psum rows -> invc cols ----
            # sq = centered^2 (x on DVE, y on ACT), then partition-sum via
            # ones-weights matmul into [1, TW] psum rows; rows go to DRAM
            # and come back transposed as [128, NB] columns.
            nrow = sm.tile([1, HW], F32, tag="nrow")
            for ti, src in ((0, xc), (1, yn)):
                sqs = []
                for g in range(2):
                    sq = sqp.tile([128, HW], BF16, tag="sqt")
                    if ti == 0:
                        nc.vector.tensor_tensor(
                            out=sq[:, :],
                            in0=src[:, g, :].bitcast(F32),
                            in1=src[:, g, :].bitcast(F32), op=ALU.mult)
                    else:
                        nc.scalar.activation(
                            out=sq[:, :],
                            in_=src[:, g, :].bitcast(F32), func=ACTF.Square)
                    sqs.append(sq)
                for h in range(NTB):
                    pr = mmq.tile([1, TW], F32, tag="pq", name=f"pr_{ti}_{h}")
                    for g in range(2):
                        for s in range(TW // 512):
                            j0 = TW * h + 512 * s
                            nc.tensor.matmul(
                                pr[:, 512 * s : 512 * (s + 1)],
                                ones_col[:, :],
                                sqs[g][:, j0 : j0 + 512],
                                start=(g == 0), stop=(g == 1))
                    nc.scalar.activation(
                        out=nrow[0:1, TW * h : TW * (h + 1)],
                        in_=pr[0:1, :], func=ACTF.Copy)
                nc.sync.dma_start(out=scr_n[ti, :], in_=nrow[0:1, :])

            # reload transposed: scr_n [2, (r p)] -> cols [128, 2*NB]
            nsq = sm.tile([128, 2 * NB], F32, tag="nsq")
            nc.sync.dma_start(
                out=nsq[:, 0:NB],
                in_=scr_n[0, :].rearrange("(r p) -> p r", p=128))
            nc.sync.dma_start(
                out=nsq[:, NB : 2 * NB],
                in_=scr_n[1, :].rearrange("(r p) -> p r", p=128))
            norms = sm.tile([128, 2 * NB], F32, tag="norms")
            nc.scalar.activation(out=norms[:, :], in_=nsq[:, :], func=ACTF.Sqrt)
            invc = sm.tile([128, 2 * NB], F32, tag="invc")
            nc.vector.reciprocal(invc[:, :], norms[:, :])
            # invx = invc[:, 0:NB], invy = invc[:, NB:2*NB]

            # ---------------- broadcast invy along partitions -------------
            nc.sync.dma_start(
                out=scr_y[:, :].rearrange("r p -> p r"),
                in_=invc[:, NB : 2 * NB])
            invybc = bc3.tile([128, HW], F32, tag="bcast")
            bcast_src_y = bass_mod.AP(
                tensor=scr_y[:, :].tensor, offset=0, ap=[[0, 128], [1, HW]])
            nc.sync.dma_start(out=invybc[:, :], in_=bcast_src_y)

            # ---------------- y_n = y_c * invy (in place, f32r) -----------
            for g in range(2):
                nc.vector.tensor_tensor(
                    out=yn[:, g, :], in0=yn[:, g, :].bitcast(F32),
                    in1=invybc[:, :], op=ALU.mult)

            # chain constants
            invc2 = sm.tile([128, NB], F32, tag="invc2")     # 2*invx
            nc.vector.tensor_scalar(
                out=invc2[:, :], in0=invc[:, 0:NB], scalar1=2.0,
                scalar2=None, op0=ALU.mult)
            ninvc = sm.tile([128, NB], F32, tag="ninvc")     # -invx
            nc.vector.tensor_scalar(
                out=ninvc[:, :], in0=invc[:, 0:NB], scalar1=-1.0,
                scalar2=None, op0=ALU.mult)
            ones_row = sm.tile([1, 128], F32R, tag="ones_row")
            ones_row_f = sm.tile([1, 128], F32, tag="ones_row_f")
            nc.vector.memset(ones_row_f[:, :], 1.0)
            nc.vector.tensor_scalar(
                out=ones_row[:, :], in0=ones_row_f[:, :], scalar1=1.0,
                scalar2=None, op0=ALU.mult)
            c_1eps = sm.tile([128, 1], F32, tag="c_1eps")
            nc.vector.memset(c_1eps[:, :], float(1.0 + EPS))
            c_2 = sm.tile([128, 1], F32, tag="c_2")
            nc.vector.memset(c_2[:, :], 2.0)

            # ---------------- PASS 1: row max + Z -------------------------
            gacc = sm.tile([128, NB * NTB], F32, tag="gacc")
            zacc = sm.tile([128, NB * NTB], F32, tag="zacc")
            gmaxc = sm.tile([128, NB], F32, tag="gmaxc")
            reccol = sm.tile([128, NB], F32, tag="reccol")
            tmpc = sm.tile([128, NB], F32, tag="tmpc")
            ab2 = sm.tile([128, 2 * NB], F32, tag="ab2")  # alpha | b2
            bcol = sm.tile([128, NB], F32, tag="bcol")

            for r in range(NB):
                pqs = [mmq.tile([128, TW], F32, tag="pq", name=f"p1_{r}_{t}")
                       for t in range(NTB)]
                for t in range(NTB):
                    for g in range(2):
                        for s in range(TW // 512):
                            j0 = TW * t + 512 * s
                            nc.tensor.matmul(
                                pqs[t][:, 512 * s : 512 * (s + 1)],
                                xc[:, g, 128 * r : 128 * (r + 1)],
                                yn[:, g, j0 : j0 + 512],
                                start=(g == 0), stop=(g == 1))
                    nc.vector.reduce_max(
                        gacc[:, NTB * r + t : NTB * r + t + 1],
                        pqs[t][:, :], axis=mybir.AxisListType.X)
                # row stats chain for this block
                if NTB == 2:
                    nc.vector.tensor_tensor(
                        out=gmaxc[:, r : r + 1],
                        in0=gacc[:, 2 * r : 2 * r + 1],
                        in1=gacc[:, 2 * r + 1 : 2 * r + 2], op=ALU.max)
                else:
                    nc.vector.reduce_max(
                        gmaxc[:, r : r + 1],
                        gacc[:, NTB * r : NTB * (r + 1)],
                        axis=mybir.AxisListType.X)
                # tmpc = 1+eps - gmax*invx  (ACT: gmax*(-invx) + (1+eps))
                nc.scalar.activation(
                    out=tmpc[:, r : r + 1], in_=gmaxc[:, r : r + 1],
                    func=ACTF.Identity, bias=c_1eps[:, 0:1],
                    scale=ninvc[:, r : r + 1])
                nc.vector.reciprocal(reccol[:, r : r + 1], tmpc[:, r : r + 1])
                # alpha = 2*invx*reccol  (ACT: reccol * invc2)
                nc.scalar.activation(
                    out=ab2[:, r : r + 1], in_=reccol[:, r : r + 1],
                    func=ACTF.Copy, scale=invc2[:, r : r + 1])
                # b = 2 - 2*reccol  (ACT)
                nc.scalar.activation(
                    out=bcol[:, r : r + 1], in_=reccol[:, r : r + 1],
                    func=ACTF.Identity, bias=c_2[:, 0:1], scale=-2.0)
                for t in range(NTB):
                    nc.scalar.activation(
                        out=pqs[t][:, :], in_=pqs[t][:, :], func=ACTF.Exp,
                        bias=bcol[:, r : r + 1],
                        scale=ab2[:, r : r + 1],
                        accum_out=zacc[:, NTB * r + t : NTB * r + t + 1])

            # ---------------- interlude: b2 = b - lnZ; broadcasts ---------
            zsum = sm.tile([128, NB], F32, tag="zsum")
            if NTB > 1:
                nc.vector.reduce_sum(
                    zsum[:, :],
                    zacc[:, :].rearrange("p (r q) -> p r q", q=NTB),
                    axis=mybir.AxisListType.X)
            else:
                nc.vector.tensor_scalar(
                    out=zsum[:, :], in0=zacc[:, :], scalar1=1.0,
                    scalar2=None, op0=ALU.mult)
            lnz = sm.tile([128, NB], F32, tag="lnz")
            nc.scalar.activation(out=lnz[:, :], in_=zsum[:, :], func=ACTF.Ln)
            nc.vector.tensor_tensor(
                out=ab2[:, NB : 2 * NB], in0=bcol[:, :], in1=lnz[:, :],
                op=ALU.subtract)

            # alpha/b2 cols -> DRAM rows (transposing dst), then partition
            # broadcast reloads
            nc.sync.dma_start(
                out=scr_a[:, :].rearrange("r p -> p r"), in_=ab2[:, 0:NB])
            nc.sync.dma_start(
                out=scr_b[:, :].rearrange("r p -> p r"),
                in_=ab2[:, NB : 2 * NB])
            abc = bc3.tile([128, HW], F32, tag="bcast")
            bcast_src_a = bass_mod.AP(
                tensor=scr_a[:, :].tensor, offset=0, ap=[[0, 128], [1, HW]])
            nc.sync.dma_start(out=abc[:, :], in_=bcast_src_a)
            nc.sync.dma_start(
                out=nrow[0:1, :], in_=scr_b[:, :].rearrange("r p -> (r p)"))
            b2row = sm.tile([1, HW], F32R, tag="b2row")
            nc.vector.tensor_scalar(
                out=b2row[:, :], in0=nrow[0:1, :], scalar1=1.0,
                scalar2=None, op0=ALU.mult)

            # x2 = x_c * alpha  (in place, f32r)
            for g in range(2):
                nc.vector.tensor_tensor(
                    out=xc[:, g, :], in0=xc[:, g, :].bitcast(F32),
                    in1=abc[:, :], op=ALU.mult)

            # ---------------- PASS 2: fused bias + col max ----------------
            macc = sm.tile([128, NB * NTB], F32, tag="macc")
            for rb in range(NB):
                pqs = [mmq.tile([128, TW], F32, tag="pq", name=f"p2_{rb}_{t}")
                       for t in range(NTB)]
                for t in range(NTB):
                    for s in range(TW // 512):
                        j0 = TW * t + 512 * s
                        nc.tensor.matmul(
                            pqs[t][:, 512 * s : 512 * (s + 1)],
                            ones_row[:, :],
                            b2row[:, j0 : j0 + 512],
                            start=True, stop=False)
                    for g in range(2):
                        for s in range(TW // 512):
                            j0 = TW * t + 512 * s
                            nc.tensor.matmul(
                                pqs[t][:, 512 * s : 512 * (s + 1)],
                                yn[:, g, 128 * rb : 128 * (rb + 1)],
                                xc[:, g, j0 : j0 + 512],
                                start=False, stop=(g == 1))
                    nc.vector.reduce_max(
                        macc[:, NTB * rb + t : NTB * rb + t + 1],
                        pqs[t][:, :], axis=mybir.AxisListType.X)

            # ---------------- final ---------------------------------------
            mcol = sm.tile([128, NB], F32, tag="mcol")
            if NTB > 1:
                nc.vector.reduce_max(
                    mcol[:, :],
                    macc[:, :].rearrange("p (r q) -> p r q", q=NTB),
                    axis=mybir.AxisListType.X)
            else:
                mcol = macc
            expm = sm.tile([128, NB], F32, tag="expm")
            csum = sm.tile([128, 1], F32, tag="csum")
            nc.scalar.activation(
                out=expm[:, :], in_=mcol[:, :], func=ACTF.Exp,
                accum_out=csum[:, :])
            tot = sm.tile([128, 1], F32, tag="tot")
            nc.gpsimd.partition_all_reduce(
                tot[:, :], csum[:, :], channels=128,
                reduce_op=bass_isa.ReduceOp.add)
            res = sm.tile([1, 1], F32, tag="res")
            nc.vector.tensor_scalar(
                out=res[:, :], in0=tot[0:1, :], scalar1=float(1.0 / HW),
                scalar2=None, op0=ALU.mult)
            nc.sync.dma_start(out=out[:, :], in_=res[:, :])
    nc.compile()
    return nc


def _get_nc():
    if "nc" not in _cached:
        _cached["nc"] = _build()
    return _cached["nc"]


def run_device(x, y, trace=False):
    """x, y: (N, C, H, W) float32. Returns (ccx (N,), BassKernelResults)."""
    x = np.ascontiguousarray(np.asarray(x, dtype=np.float32))
    y = np.ascontiguousarray(np.asarray(y, dtype=np.float32))
    ymu = y.mean(axis=(0, 2, 3), dtype=np.float64).astype(np.float32)  # (C,)
    ymu_arr = np.ascontiguousarray(ymu.reshape(2, 128).T)  # (128, 2)
    in_maps = []
    for n in range(N):
        in_maps.append({
            "xs": np.ascontiguousarray(x[n].reshape(C, HW)),
            "ys": np.ascontiguousarray(y[n].reshape(C, HW)),
            "ymu": ymu_arr,
        })
    nc = _get_nc()
    res = run_bass_kernel_spmd(nc, in_maps, core_ids=list(range(N)), trace=trace)
    ccx = np.array([res.results[n]["out"][0, 0] for n in range(N)], dtype=np.float32)
    return ccx, res


def kernel(x, y):
    ccx, _ = run_device(x, y)
    loss = float(np.mean(-np.log(ccx.astype(np.float64) + EPS)))
    return np.float32(loss)


if __name__ == "__main__":
    rng = np.random.default_rng(0)
    x = rng.standard_normal((N, C, H, W), dtype=np.float32)
    y = rng.standard_normal((N, C, H, W), dtype=np.float32)
    print("loss:", kernel(x, y))


# revision 16
# speedup vs baseline: 1.0295x; 1.0215x over previous
"""CCX loss kernel for Trainium2 (8 NeuronCores, data-parallel over batch).

Math (per batch element n, with C=256 channels, HW=64*64=4096 pixels):
  y_mu[c]   = mean over (n, h, w) of y            (host, tiny)
  x_c = x - y_mu ; y_c = y - y_mu                 (device)
  x_n = x_c/||x_c||_C ; y_n = y_c/||y_c||_C       (device)
  s[i,j]    = sum_c x_n[c,i] y_n[c,j]             (device matmul, f32r)
  d = 1-s ; dt = d/(dmin_i+eps) ; w = exp((1-dt)/0.5)
  ccx_ij = w/sum_j w ; ccx_n = mean_j max_i ccx_ij
  loss = mean_n -log(ccx_n + eps)                 (host, 8 scalars)

Key identities used on device:
  w_ij = exp(s*a_i + b_i),  a_i = 2/(dmin_i+eps), b_i = 2-a_i
  s*a_i = G*alpha_i with G = x_c^T y_n (unnormalized-x matmul),
          alpha_i = a_i/||x_c[:,i]||
  max_i ccx_ij = exp(max_i (G^T[j,i]*alpha_i + (b_i - lnZ_i)))
  -> pass 2: K=1 ones matmul seeds psum with the bias row b2, the two
     K=128 matmuls accumulate G^T*alpha, one reduce_max per tile.

Perf structure (TRN2):
  - psum tiles are TW-wide; matmuls write 512-wide slices.
  - y-path first in the preamble (yn gates pass1); x-norm path overlaps
    pass1 start.  x loads on the gpsimd DMA queue, y on the SP queue.
  - pass1 per block: matmuls (K-group outer, shared LDWEIGHTS),
    reduce_max per tile, all-DVE row-stats chain (no cross-engine
    ping-pong on the critical path), exp in-place with Z accumulation.
  - pass2: bias seed + matmuls + reduce_max per tile; x2 scaling is
    chunked so early pass2 blocks start before the scaling finishes.
"""

import os
import sys

import numpy as np

sys.path.insert(0, "/opt/trn_rl_repo")
os.environ.setdefault("JAX_PLATFORMS", "axon")

import concourse.mybir as mybir
import concourse.tile as tile
from concourse import bacc, bass_isa
from concourse.bass_utils import run_bass_kernel_spmd

N, C, H, W = 8, 256, 64, 64
HW = H * W          # 4096
EPS = 1e-6
F32 = mybir.dt.float32
F32R = mybir.dt.float32r
BF16 = mybir.dt.bfloat16
ALU = mybir.AluOpType
ACTF = mybir.ActivationFunctionType

TW = int(os.environ.get("BASS_TW", "1024"))    # psum tile width
NTB = HW // TW                                  # tiles per block row
NBUF = 16384 // (TW * 4)                        # psum ring generations
NB = HW // 128                                  # 32 row/col blocks
NS = TW // 512                                  # 512-slices per tile

_cached = {}


def _build():
    nc = bacc.Bacc(None, target_bir_lowering=False, debug=True)
    xs = nc.dram_tensor("xs", [C, HW], F32, kind="ExternalInput")
    ys = nc.dram_tensor("ys", [C, HW], F32, kind="ExternalInput")
    ymu = nc.dram_tensor("ymu", [128, 2], F32, kind="ExternalInput")
    out = nc.dram_tensor("out", [1, 1], F32, kind="ExternalOutput")
    scr_nx = nc.dram_tensor("scr_nx", [1, HW], F32)  # x normsq row
    scr_ny = nc.dram_tensor("scr_ny", [1, HW], F32)  # y normsq row
    scr_y = nc.dram_tensor("scr_y", [NB, 128], F32)
    scr_a = nc.dram_tensor("scr_a", [NB, 128], F32)
    scr_b = nc.dram_tensor("scr_b", [NB, 128], F32)

    import concourse.bass as bass_mod

    with tile.TileContext(nc) as tc:
        with (
            tc.tile_pool(name="big", bufs=1) as big,
            tc.tile_pool(name="bc3", bufs=1) as bc3,
            tc.tile_pool(name="sq", bufs=2) as sqp,
            tc.tile_pool(name="small", bufs=1) as sm,
            tc.tile_pool(name="mmq", bufs=NBUF, space="PSUM") as mmq,
        ):
            # ---------------- load (y on SP queue, x on gpsimd queue) -----
            x = big.tile([128, 2, HW], F32, tag="x")
            y = big.tile([128, 2, HW], F32, tag="y")
            xc = big.tile([128, 2, HW], F32R, tag="xc")
            yn = big.tile([128, 2, HW], F32R, tag="yn")
            ymu_sb = sm.tile([128, 2], F32, tag="ymu")
            nc.sync.dma_start(out=ymu_sb[:, :], in_=ymu[:, :])
            nc.sync.dma_start(out=y[:, :, :],
                              in_=ys.rearrange("(g p) j -> p g j", p=128))
            nc.gpsimd.dma_start(out=x[:, :, :],
                                in_=xs.rearrange("(g p) j -> p g j", p=128))

            ones_col = sm.tile([128, 1], BF16, tag="ones_col")
            nc.vector.memset(ones_col[:, :], 1.0)
            negymu = sm.tile([128, 2], F32, tag="negymu")
            nc.vector.tensor_scalar(
                out=negymu[:, :], in0=ymu_sb[:, :], scalar1=-1.0,
                scalar2=None, op0=ALU.mult)
            ones_row = sm.tile([1, 128], F32R, tag="ones_row")
            ones_row_f = sm.tile([1, 128], F32, tag="ones_row_f")
            nc.vector.memset(ones_row_f[:, :], 1.0)
            nc.vector.tensor_scalar(
                out=ones_row[:, :], in0=ones_row_f[:, :], scalar1=1.0,
                scalar2=None, op0=ALU.mult)

            # ---------------- y path: center, sumsq, invy bcast, yn ------
            nrow = sm.tile([1, HW], F32, tag="nrow")

            def sumsq_rows(src, scr, use_act):
                sqs = []
                for g in range(2):
                    sq = sqp.tile([128, HW], BF16, tag="sqt")
                    if use_act:
                        nc.scalar.activation(
                            out=sq[:, :], in_=src[:, g, :].bitcast(F32),
                            func=ACTF.Square)
                    else:
                        nc.vector.tensor_tensor(
                            out=sq[:, :], in0=src[:, g, :].bitcast(F32),
                            in1=src[:, g, :].bitcast(F32), op=ALU.mult)
                    sqs.append(sq)
                for h in range(HW // TW):
                    pr = mmq.tile([1, TW], F32, tag="pq",
                                  name=f"pr_{scr.name}_{h}")
                    for g in range(2):
                        for s in range(NS):
                            j0 = TW * h + 512 * s
                            nc.tensor.matmul(
                                pr[:, 512 * s : 512 * (s + 1)],
                                ones_col[:, :],
                                sqs[g][:, j0 : j0 + 512],
                                start=(g == 0), stop=(g == 1))
                    nc.scalar.activation(
                        out=nrow[0:1, TW * h : TW * (h + 1)],
                        in_=pr[0:1, :], func=ACTF.Copy)
                nc.sync.dma_start(out=scr[0, :], in_=nrow[0:1, :])

            # y center (ACT) while x still loading
            for g in range(2):
                nc.scalar.activation(
                    out=yn[:, g, :], in_=y[:, g, :],
                    func=ACTF.Identity, bias=negymu[:, g : g + 1], scale=1.0)
            sumsq_rows(yn, scr_ny, use_act=True)

            # invy cols and partition-broadcast
            nsqy = sm.tile([128, NB], F32, tag="nsqy")
            nc.sync.dma_start(
                out=nsqy[:, :], in_=scr_ny[0, :].rearrange("(r p) -> p r", p=128))
            normy = sm.tile([128, NB], F32, tag="normy")
            nc.scalar.activation(out=normy[:, :], in_=nsqy[:, :], func=ACTF.Sqrt)
            invy = sm.tile([128, NB], F32, tag="invy")
            nc.vector.reciprocal(invy[:, :], normy[:, :])
            nc.sync.dma_start(
                out=scr_y[:, :].rearrange("r p -> p r"), in_=invy[:, :])
            invybc = bc3.tile([128, HW], F32, tag="bcast")
            bcast_src_y = bass_mod.AP(
                tensor=scr_y[:, :].tensor, offset=0, ap=[[0, 128], [1, HW]])
            nc.sync.dma_start(out=invybc[:, :], in_=bcast_src_y)

            # y_n = y_c * invy (in place, f32r)
            for g in range(2):
                nc.vector.tensor_tensor(
                    out=yn[:, g, :], in0=yn[:, g, :].bitcast(F32),
                    in1=invybc[:, :], op=ALU.mult)

            # ---------------- x path (overlaps pass1 head) ----------------
            for g in range(2):
                nc.vector.tensor_scalar(
                    out=xc[:, g, :], in0=x[:, g, :],
                    scalar1=ymu_sb[:, g : g + 1], scalar2=None,
                    op0=ALU.subtract)
            sumsq_rows(xc, scr_nx, use_act=True)
            nsqx = sm.tile([128, NB], F32, tag="nsqx")
            nc.sync.dma_start(
                out=nsqx[:, :], in_=scr_nx[0, :].rearrange("(r p) -> p r", p=128))
            normx = sm.tile([128, NB], F32, tag="normx")
            nc.scalar.activation(out=normx[:, :], in_=nsqx[:, :], func=ACTF.Sqrt)
            invx = sm.tile([128, NB], F32, tag="invx")
            nc.vector.reciprocal(invx[:, :], normx[:, :])
            invx2 = sm.tile([128, NB], F32, tag="invx2")     # 2*invx
            nc.vector.tensor_scalar(
                out=invx2[:, :], in0=invx[:, :], scalar1=2.0,
                scalar2=None, op0=ALU.mult)
            ninvx = sm.tile([128, NB], F32, tag="ninvx")     # -invx
            nc.vector.tensor_scalar(
                out=ninvx[:, :], in0=invx[:, :], scalar1=-1.0,
                scalar2=None, op0=ALU.mult)

            # ---------------- PASS 1: row max + Z -------------------------
            gacc = sm.tile([128, NB * NTB], F32, tag="gacc")
            zacc = sm.tile([128, NB * NTB], F32, tag="zacc")
            gmaxc = sm.tile([128, NB], F32, tag="gmaxc")
            reccol = sm.tile([128, NB], F32, tag="reccol")
            tmpc = sm.tile([128, NB], F32, tag="tmpc")
            ab2 = sm.tile([128, 2 * NB], F32, tag="ab2")  # alpha | b2
            bcol = sm.tile([128, NB], F32, tag="bcol")

            for r in range(NB):
                pqs = [mmq.tile([128, TW], F32, tag="pq", name=f"p1_{r}_{t}")
                       for t in range(NTB)]
                for t in range(NTB):
                    for g in range(2):
                        for s in range(NS):
                            j0 = TW * t + 512 * s
                            nc.tensor.matmul(
                                pqs[t][:, 512 * s : 512 * (s + 1)],
                                xc[:, g, 128 * r : 128 * (r + 1)],
                                yn[:, g, j0 : j0 + 512],
                                start=(g == 0), stop=(g == 1))
                    nc.vector.reduce_max(
                        gacc[:, NTB * r + t : NTB * r + t + 1],
                        pqs[t][:, :], axis=mybir.AxisListType.X)
                # all-DVE row stats chain (back-to-back, same engine)
                nc.vector.reduce_max(
                    gmaxc[:, r : r + 1],
                    gacc[:, NTB * r : NTB * (r + 1)],
                    axis=mybir.AxisListType.X)
                # tmpc = 1+eps - gmax*invx
                nc.vector.tensor_scalar(
                    out=tmpc[:, r : r + 1], in0=gmaxc[:, r : r + 1],
                    scalar1=ninvx[:, r : r + 1], scalar2=float(1.0 + EPS),
                    op0=ALU.mult, op1=ALU.add)
                nc.vector.reciprocal(reccol[:, r : r + 1], tmpc[:, r : r + 1])
                # alpha = 2*invx*reccol
                nc.vector.tensor_scalar(
                    out=ab2[:, r : r + 1], in0=reccol[:, r : r + 1],
                    scalar1=invx2[:, r : r + 1], scalar2=None, op0=ALU.mult)
                # b = 2 - 2*reccol
                nc.vector.tensor_scalar(
                    out=bcol[:, r : r + 1], in0=reccol[:, r : r + 1],
                    scalar1=-2.0, scalar2=2.0, op0=ALU.mult, op1=ALU.add)
                for t in range(NTB):
                    nc.scalar.activation(
                        out=pqs[t][:, :], in_=pqs[t][:, :], func=ACTF.Exp,
                        bias=bcol[:, r : r + 1],
                        scale=ab2[:, r : r + 1],
                        accum_out=zacc[:, NTB * r + t : NTB * r + t + 1])

            # ---------------- interlude: b2 = b - lnZ; broadcasts ---------
            zsum = sm.tile([128, NB], F32, tag="zsum")
            nc.vector.reduce_sum(
                zsum[:, :],
                zacc[:, :].rearrange("p (r q) -> p r q", q=NTB),
                axis=mybir.AxisListType.X)
            lnz = sm.tile([128, NB], F32, tag="lnz")
            nc.scalar.activation(out=lnz[:, :], in_=zsum[:, :], func=ACTF.Ln)
            nc.vector.tensor_tensor(
                out=ab2[:, NB : 2 * NB], in0=bcol[:, :], in1=lnz[:, :],
                op=ALU.subtract)

            # alpha -> bcast via SP queue; b2 -> row via gpsimd queue
            nc.sync.dma_start(
                out=scr_a[:, :].rearrange("r p -> p r"), in_=ab2[:, 0:NB])
            nc.gpsimd.dma_start(
                out=scr_b[:, :].rearrange("r p -> p r"),
                in_=ab2[:, NB : 2 * NB])
            abc = bc3.tile([128, HW], F32, tag="bcast")
            bcast_src_a = bass_mod.AP(
                tensor=scr_a[:, :].tensor, offset=0, ap=[[0, 128], [1, HW]])
            nc.sync.dma_start(out=abc[:, :], in_=bcast_src_a)
            nc.gpsimd.dma_start(
                out=nrow[0:1, :], in_=scr_b[:, :].rearrange("r p -> (r p)"))
            b2row = sm.tile([1, HW], F32R, tag="b2row")
            nc.vector.tensor_scalar(
                out=b2row[:, :], in0=nrow[0:1, :], scalar1=1.0,
                scalar2=None, op0=ALU.mult)

            # x2 = x_c * alpha (in place, f32r), chunked so pass2 can start
            XCH = 1024
            for j0 in range(0, HW, XCH):
                for g in range(2):
                    nc.vector.tensor_tensor(
                        out=xc[:, g, j0 : j0 + XCH],
                        in0=xc[:, g, j0 : j0 + XCH].bitcast(F32),
                        in1=abc[:, j0 : j0 + XCH], op=ALU.mult)

            # ---------------- PASS 2: bias seed + col max -----------------
            macc = sm.tile([128, NB * NTB], F32, tag="macc")
            for rb in range(NB):
                pqs = [mmq.tile([128, TW], F32, tag="pq", name=f"p2_{rb}_{t}")
                       for t in range(NTB)]
                for t in range(NTB):
                    for s in range(NS):
                        j0 = TW * t + 512 * s
                        nc.tensor.matmul(
                            pqs[t][:, 512 * s : 512 * (s + 1)],
                            ones_row[:, :],
                            b2row[:, j0 : j0 + 512],
                            start=True, stop=False)
                    for g in range(2):
                        for s in range(NS):
                            j0 = TW * t + 512 * s
                            nc.tensor.matmul(
                                pqs[t][:, 512 * s : 512 * (s + 1)],
                                yn[:, g, 128 * rb : 128 * (rb + 1)],
                                xc[:, g, j0 : j0 + 512],
                                start=False, stop=(g == 1))
                    nc.vector.reduce_max(
                        macc[:, NTB * rb + t : NTB * rb + t + 1],
                        pqs[t][:, :], axis=mybir.AxisListType.X)

            # ---------------- final ---------------------------------------
            mcol = sm.tile([128, NB], F32, tag="mcol")
            nc.vector.reduce_max(
                mcol[:, :],
                macc[:, :].rearrange("p (r q) -> p r q", q=NTB),
                axis=mybir.AxisListType.X)
            expm = sm.tile([128, NB], F32, tag="expm")
            csum = sm.tile([128, 1], F32, tag="csum")
            nc.scalar.activation(
                out=expm[:, :], in_=mcol[:, :], func=ACTF.Exp,
                accum_out=csum[:, :])
            tot = sm.tile([128, 1], F32, tag="tot")
            nc.gpsimd.partition_all_reduce(
                tot[:, :], csum[:, :], channels=128,
                reduce_op=bass_isa.ReduceOp.add)
            res = sm.tile([1, 1], F32, tag="res")
            nc.vector.tensor_scalar(
                out=res[:, :], in0=tot[0:1, :], scalar1=float(1.0 / HW),
                scalar2=None, op0=ALU.mult)
            nc.sync.dma_start(out=out[:, :], in_=res[:, :])
    nc.compile()
    return nc


def _get_nc():
    if "nc" not in _cached:
        _cached["nc"] = _build()
    return _cached["nc"]


def run_device(x, y, trace=False):
    """x, y: (N, C, H, W) float32. Returns (ccx (N,), BassKernelResults)."""
    x = np.ascontiguousarray(np.asarray(x, dtype=np.float32))
    y = np.ascontiguousarray(np.asarray(y, dtype=np.float32))
    ymu = y.mean(axis=(0, 2, 3), dtype=np.float64).astype(np.float32)  # (C,)
    ymu_arr = np.ascontiguousarray(ymu.reshape(2, 128).T)  # (128, 2)
    in_maps = []
    for n in range(N):
        in_maps.append({
            "xs": np.ascontiguousarray(x[n].reshape(C, HW)),
            "ys": np.ascontiguousarray(y[n].reshape(C, HW)),
            "ymu": ymu_arr,
        })
    nc = _get_nc()
    res = run_bass_kernel_spmd(nc, in_maps, core_ids=list(range(N)), trace=trace)
    ccx = np.array([res.results[n]["out"][0, 0] for n in range(N)], dtype=np.float32)
    return ccx, res


def kernel(x, y):
    ccx, _ = run_device(x, y)
    loss = float(np.mean(-np.log(ccx.astype(np.float64) + EPS)))
    return np.float32(loss)


if __name__ == "__main__":
    rng = np.random.default_rng(0)
    x = rng.standard_normal((N, C, H, W), dtype=np.float32)
    y = rng.standard_normal((N, C, H, W), dtype=np.float32)
    print("loss:", kernel(x, y))
